# revision 8
# baseline (speedup 1.0000x reference)
"""Trainium2 Bass kernel for AffinityNeuralNetworkCliffNetMONN.

Sharding: data-parallel over the B=128 graphs; each of the 8 NeuronCores
processes 16 whole graphs. Weights are replicated. Host pre-transposes node
features to feature-major layout and packs weights; device computes the full
network; host concatenates the per-core [16] affinity outputs into [B, 1].

Device layouts (per core, G=16 graphs):
  - protT [128, G*1024], compT [128, G*64]: feature-major node features (f32r)
  - activations feature-major [H=128 partitions, nodes free]
  - pw packed 2-graphs-per-128-partitions: pw2 [128, 8*1024]
  - pwT (transposed pairwise map) [128, G*512], chunk-major
  - node softmax on [16, nodes] tiles (graph per partition); exp rows are
    broadcast back to 128 partitions via a DRAM round-trip DMA.
Matmuls run in float32r (full PE rate, ~1.6e-4 matmul precision). The
reference's sigmoid is computed as 0.5*tanh(0.5x)+0.5 so the whole kernel
stays in the exp/tanh/lrelu activation-table set (no table switches).
"""
import sys

sys.path.insert(0, '/opt/trn_rl_repo')

import numpy as np

B, LC, LP, H, D = 128, 64, 1024, 128, 3
NCORES = 8
G = B // NCORES  # 16 graphs per core

_cache = {}
DEBUG = False


def _build():
    from contextlib import ExitStack
    import concourse.bass as bass
    from concourse import bacc
    import concourse.tile as tile
    from concourse import mybir
    from concourse.masks import make_identity

    F32 = mybir.dt.float32
    F32R = mybir.dt.float32r
    AF = mybir.ActivationFunctionType
    OP = mybir.AluOpType
    AX = mybir.AxisListType

    # ---- bias/vector pack column map (fp32 pack, one [128,1] column each)
    BC = {}
    col = 0
    for name in ['bpc', 'bpp', 'bca', 'bpa']:
        BC[name] = col; col += 1
    for name in ['bc2p', 'bp2c', 'bmc1', 'bmp1', 'bhc0', 'bhp0',
                 'Whc1', 'Whp1', 'bhp1', 'bhc1']:
        for i in range(D):
            BC[f'{name}{i}'] = col; col += 1
    for name in ['bh_r', 'bh_z', 'bih_n', 'bhh_n', 'b_out']:
        BC[name] = col; col += 1
    NB = col

    # ---- matmul weight pack column map (f32r pack)
    WC = {}
    wcol = 0
    for name in ['Wpc', 'Wpp', 'Wca', 'Wpa']:
        WC[name] = wcol; wcol += 128
    for name in ['Wc2p', 'Wp2c', 'Wmc1', 'Wmp1', 'Whc0', 'Whp0']:
        for i in range(D):
            WC[f'{name}{i}'] = wcol; wcol += 128
    WC['WihT'] = wcol; wcol += 3 * 128
    WC['WhhT'] = wcol; wcol += 3 * 128
    for i in range(D):
        WC[f'bc2p_row{i}'] = wcol; wcol += 128   # [1,128] row bias (c_pre)
        WC[f'bp2c_row{i}'] = wcol; wcol += 512   # [1,512] = 4 replicas (p_pre)
    NW = wcol

    nc = bacc.Bacc("TRN2", target_bir_lowering=False, debug=False)

    protT_d = nc.dram_tensor("protT", (128, G * 1024), F32R, kind="ExternalInput").ap()
    compT_d = nc.dram_tensor("compT", (128, G * 64), F32R, kind="ExternalInput").ap()
    Wr_d = nc.dram_tensor("Wr", (128, NW), F32R, kind="ExternalInput").ap()
    Bf_d = nc.dram_tensor("Bf", (128, NB), F32, kind="ExternalInput").ap()
    Wo_d = nc.dram_tensor("Wo", (128, 128), F32, kind="ExternalInput").ap()
    out_d = nc.dram_tensor("out", (1, G), F32, kind="ExternalOutput").ap()
    dbg = {}
    if DEBUG:
        for nm, shp in [("d_m0", (128, G)), ("d_pw0", (64, 1024)),
                        ("d_pw1", (64, 1024)), ("d_pwT0", (128, 512)),
                        ("d_ep0", (G, 1024)), ("d_ec0", (G, 64)),
                        ("d_cf0", (128, G)), ("d_pf0", (128, G)),
                        ("d_m1", (128, G)), ("d_wallp0", (128, G)),
                        ("d_hp0T_g0", (128, 1024)), ("d_c2p_g0", (128, 1024)),
                        ("d_qp_g0", (128, 1024)), ("d_ppre_g0", (128, 1024)),
                        ("d_p2c_g0", (128, 64)), ("d_rz0", (1, G))]:
            dbg[nm] = nc.dram_tensor(nm, shp, F32, kind="ExternalOutput").ap()

    ep_scr = nc.dram_tensor("ep_scr", (G, 1024), F32, kind="Internal").ap()
    ec_scr = nc.dram_tensor("ec_scr", (G, 64), F32, kind="Internal").ap()
    rz_scr = nc.dram_tensor("rz_scr", (1, G), F32, kind="Internal").ap()
    cf_scr = nc.dram_tensor("cf_scr", (G, 128), F32, kind="Internal").ap()
    pf_scr = nc.dram_tensor("pf_scr", (G, 128), F32, kind="Internal").ap()

    with tile.TileContext(nc) as tc, ExitStack() as top:
        persist = top.enter_context(tc.tile_pool(name="persist", bufs=1))
        pbig = top.enter_context(tc.tile_pool(name="pbig", bufs=2, space="PSUM"))
        psmall = top.enter_context(tc.tile_pool(name="psmall", bufs=1, space="PSUM"))

        # ---------------- constants / weights ----------------
        WrS = persist.tile([128, NW], F32R)
        BfS = persist.tile([128, NB], F32)
        WoS = persist.tile([128, 128], F32)
        nc.sync.dma_start(out=WrS, in_=Wr_d)
        nc.sync.dma_start(out=BfS, in_=Bf_d)
        nc.sync.dma_start(out=WoS, in_=Wo_d)

        ident = persist.tile([128, 128], F32)
        make_identity(nc, ident)

        ones_row = persist.tile([1, 128], F32)
        nc.vector.memset(ones_row, 1.0)
        ones_col = persist.tile([128, 1], F32)
        nc.vector.memset(ones_col, 1.0)
        ones_row_r = persist.tile([1, 128], F32R)
        nc.vector.tensor_copy(ones_row_r, ones_row)

        # OnesBlk[h, g, m] = 1 iff g == m  (f32r selector for score matmuls)
        OnesBlkF = persist.tile([128, G, G], F32)
        nc.vector.memset(OnesBlkF, 0.0)
        diag = bass.AP(tensor=OnesBlkF.tensor, offset=OnesBlkF.offset,
                       ap=[OnesBlkF.ap[0], [G + 1, G]])
        nc.vector.memset(diag, 1.0)
        OnesBlk = persist.tile([128, G, G], F32R)
        nc.vector.tensor_copy(OnesBlk, OnesBlkF)

        def W(name):
            return WrS[:, WC[name]:WC[name] + 128]

        def Wih(j):
            return WrS[:, WC['WihT'] + j * 128:WC['WihT'] + (j + 1) * 128]

        def Whh(j):
            return WrS[:, WC['WhhT'] + j * 128:WC['WhhT'] + (j + 1) * 128]

        def bias(name):
            return BfS[:, BC[name]:BC[name] + 1]

        def bias16(name):
            return BfS[0:16, BC[name]:BC[name] + 1]

        # ---------------- persistent activations ----------------
        peT = persist.tile([128, G * 1024], F32R)   # lrelu prot features
        ceT = persist.tile([128, G * 64], F32R)     # lrelu comp features
        pw2 = persist.tile([128, (G // 2) * 1024], F32R)  # pairwise, 2-graph packed
        pwT = persist.tile([128, G * 512], F32R)    # pairwise transposed, chunked
        csum = persist.tile([128, G], F32)
        psums = persist.tile([128, G], F32)
        mT = persist.tile([128, G], F32R)
        cfT = persist.tile([128, G], F32)
        pfT = persist.tile([128, G], F32)
        rzrow = persist.tile([1, G], F32)
        dump = persist.tile([128, 1024], F32)
        dumpc = persist.tile([128, 64], F32)

        # ================= PREP PHASE =================
        with ExitStack() as prep:
            sprep = prep.enter_context(tc.tile_pool(name="sprep", bufs=1))
            strm = prep.enter_context(tc.tile_pool(name="strm", bufs=2))
            pprep = prep.enter_context(tc.tile_pool(name="pprep", bufs=1, space="PSUM"))

            compT = sprep.tile([128, G * 64], F32R, tag="compT")
            nc.sync.dma_start(out=compT, in_=compT_d)

            # pcT / ceT for all graphs  (2x N=512 matmuls each)
            pcT = sprep.tile([128, G * 64], F32R, tag="pcT")
            pc_ps = pbig.tile([128, 1024], F32, tag="pb")
            ce_ps = pbig.tile([128, 1024], F32, tag="pb")
            for c in range(2):
                sl = slice(c * 512, (c + 1) * 512)
                nc.tensor.matmul(pc_ps[:, sl], W('Wpc'), compT[:, sl],
                                 start=True, stop=True)
                nc.tensor.matmul(ce_ps[:, sl], W('Wca'), compT[:, sl],
                                 start=True, stop=True)
            nc.scalar.activation(pcT, pc_ps, AF.Prelu, bias=bias('bpc'), alpha=0.1)
            for g in range(G):
                sl = slice(g * 64, (g + 1) * 64)
                nc.scalar.activation(ceT[:, sl], ce_ps[:, sl], AF.Prelu,
                                     bias=bias('bca'), alpha=0.1,
                                     accum_out=csum[:, g:g + 1])

            pwps = None
            for g in range(G):
                psl = slice(g * 1024, (g + 1) * 1024)
                protT = strm.tile([128, 1024], F32R, tag="protT")
                nc.sync.dma_start(out=protT, in_=protT_d[:, psl])

                pp_ps = pbig.tile([128, 1024], F32, tag="pb")
                pe_ps = pbig.tile([128, 1024], F32, tag="pb")
                for c in range(2):
                    sl = slice(c * 512, (c + 1) * 512)
                    nc.tensor.matmul(pp_ps[:, sl], W('Wpp'), protT[:, sl],
                                     start=True, stop=True)
                    nc.tensor.matmul(pe_ps[:, sl], W('Wpa'), protT[:, sl],
                                     start=True, stop=True)
                ppT = strm.tile([128, 1024], F32R, tag="ppT")
                nc.scalar.activation(ppT, pp_ps, AF.Prelu, bias=bias('bpp'), alpha=0.1)
                nc.scalar.activation(peT[:, psl], pe_ps, AF.Prelu, bias=bias('bpa'),
                                     alpha=0.1, accum_out=psums[:, g:g + 1])

                # pairwise map z = pcT_g.T @ ppT  ->  rows (g%2)*64 of pair psum
                half = (g % 2) * 64
                if g % 2 == 0:
                    pwps = pprep.tile([128, 1024], F32, tag="pwps")
                for c in range(2):
                    sl = slice(c * 512, (c + 1) * 512)
                    # f32r matmuls cannot target dst partition 64: use fp32
                    # there (operand values are already f32r-rounded).
                    if half:
                        nc.tensor.matmul(pwps[half:half + 64, sl],
                                         pcT[:, g * 64:(g + 1) * 64].bitcast(F32),
                                         ppT[:, sl].bitcast(F32),
                                         start=True, stop=True)
                    else:
                        nc.tensor.matmul(pwps[half:half + 64, sl],
                                         pcT[:, g * 64:(g + 1) * 64], ppT[:, sl],
                                         start=True, stop=True)
                if g % 2 == 1:
                    k = g // 2
                    ksl = slice(k * 1024, (k + 1) * 1024)
                    # sigmoid(z) = 0.5*tanh(0.5 z) + 0.5
                    tsig = strm.tile([128, 1024], F32, tag="tsig")
                    nc.scalar.activation(tsig, pwps, AF.Tanh, scale=0.5)
                    nc.vector.tensor_scalar(out=pw2[:, ksl], in0=tsig,
                                            scalar1=0.5, scalar2=0.5,
                                            op0=OP.mult, op1=OP.add)

            # transpose pw -> pwT   (per graph: 8 PE transposes of [64,128])
            for g in range(G):
                half = (g % 2) * 64
                k = g // 2
                trp = pprep.tile([128, 8, 64], F32, tag="trp")
                for j in range(8):
                    src = pw2[half:half + 64,
                              k * 1024 + j * 128:k * 1024 + (j + 1) * 128]
                    nc.tensor.transpose(
                        trp[:, j], src.bitcast(F32),
                        ident[half:half + 64, half:half + 64])
                nc.vector.tensor_copy(
                    pwT[:, g * 512:(g + 1) * 512],
                    trp.rearrange("p a b -> p (a b)"))

            # m0 = mean(ce) * mean(pe)
            m0t = sprep.tile([128, G], F32, tag="m0t")
            nc.vector.tensor_tensor(out=m0t, in0=csum, in1=psums, op=OP.mult)
            nc.vector.tensor_scalar(out=mT, in0=m0t, scalar1=1.0 / (64.0 * 1024.0),
                                    scalar2=None, op0=OP.mult)
            if DEBUG:
                nc.sync.dma_start(out=dbg["d_m0"], in_=mT.bitcast(F32))
                nc.sync.dma_start(out=dbg["d_pw0"], in_=pw2[0:64, 0:1024].bitcast(F32))
                nc.sync.dma_start(out=dbg["d_pw1"], in_=pw2[64:128, 0:1024].bitcast(F32))
                nc.sync.dma_start(out=dbg["d_pwT0"], in_=pwT[:, 0:512].bitcast(F32))

        # ================= MESSAGE-PASSING LOOP =================
        with ExitStack() as loop:
            sloop = loop.enter_context(tc.tile_pool(name="sloop", bufs=5))
            sep = loop.enter_context(tc.tile_pool(name="sep", bufs=2))
            sc64 = loop.enter_context(tc.tile_pool(name="sc64", bufs=4))
            spair = loop.enter_context(tc.tile_pool(name="spair", bufs=4))
            ssm = loop.enter_context(tc.tile_pool(name="ssm", bufs=4))
            shc = loop.enter_context(tc.tile_pool(name="shc", bufs=1))
            pS = loop.enter_context(tc.tile_pool(name="pS", bufs=1, space="PSUM"))

            for i in range(D):
                # ---- m-dependent folded score weights
                mc_ps = psmall.tile([128, G], F32, tag="ps128")
                nc.tensor.matmul(mc_ps, W(f'Wmc1{i}'), mT, start=True, stop=True)
                mcT = ssm.tile([128, G], F32, tag="mcT")
                nc.scalar.activation(mcT, mc_ps, AF.Tanh, bias=bias(f'bmc1{i}'))
                wallc = ssm.tile([128, G], F32, tag="wallc")
                nc.vector.tensor_scalar(out=wallc, in0=mcT, scalar1=bias(f'Whc1{i}'),
                                        scalar2=None, op0=OP.mult)

                mp_ps = psmall.tile([128, G], F32, tag="ps128")
                nc.tensor.matmul(mp_ps, W(f'Wmp1{i}'), mT, start=True, stop=True)
                mpT = ssm.tile([128, G], F32, tag="mpT")
                nc.scalar.activation(mpT, mp_ps, AF.Tanh, bias=bias(f'bmp1{i}'))
                wallp = ssm.tile([128, G], F32, tag="wallp")
                nc.vector.tensor_scalar(out=wallp, in0=mpT, scalar1=bias(f'Whp1{i}'),
                                        scalar2=None, op0=OP.mult)

                # ---- hc0T for all graphs (feature-major)
                hc_ps = pbig.tile([128, 1024], F32, tag="pb")
                for c in range(2):
                    sl = slice(c * 512, (c + 1) * 512)
                    nc.tensor.matmul(hc_ps[:, sl], W(f'Whc0{i}'), ceT[:, sl],
                                     start=True, stop=True)
                hc0T = shc.tile([128, G * 64], F32, tag="hc0T")
                nc.scalar.activation(hc0T, hc_ps, AF.Tanh, bias=bias(f'bhc0{i}'))

                S_p = pS.tile([16, 1024], F32, tag="Sp")
                S_c = pS.tile([16, 64], F32, tag="Sc")

                cpn = None
                for g in range(G):
                    half = (g % 2) * 64
                    k = g // 2

                    # ---- c_pre natural, 2-graph packed [128(c), 128(h)]
                    if g % 2 == 0:
                        cp_ps = psmall.tile([128, 128], F32, tag="ps128")
                        # natural layout [node, h]: bias lives on the free dim;
                        # seed psum with ones x bias_row, then accumulate.
                        bc_row = WrS[0:1, WC[f'bc2p_row{i}']:WC[f'bc2p_row{i}'] + 128]
                        nc.tensor.matmul(cp_ps, ones_row_r, bc_row,
                                         start=True, stop=False)
                        for hh in (0, 64):
                            gg = 2 * k + (hh // 64)
                            if hh:
                                nc.tensor.matmul(
                                    cp_ps[hh:hh + 64, :],
                                    ceT[:, gg * 64:(gg + 1) * 64].bitcast(F32),
                                    W(f'Wc2p{i}').bitcast(F32),
                                    start=False, stop=(hh == 64),
                                    skip_group_check=True)
                            else:
                                nc.tensor.matmul(cp_ps[hh:hh + 64, :],
                                                 ceT[:, gg * 64:(gg + 1) * 64],
                                                 W(f'Wc2p{i}'), start=False,
                                                 stop=False, skip_group_check=True)
                        cpn = spair.tile([128, 128], F32R, tag="cpn")
                        nc.scalar.activation(cpn, cp_ps, AF.Tanh)

                    # p_pre natural: 8 chunk matmuls peT_chunk.T @ Wp2c
                    ppre_ps = pbig.tile([128, 1024], F32, tag="pb")
                    bp_row = WrS[0:1, WC[f'bp2c_row{i}']:WC[f'bp2c_row{i}'] + 512]
                    for c in range(2):
                        nc.tensor.matmul(ppre_ps[:, c * 512:(c + 1) * 512],
                                         ones_row_r, bp_row, start=True, stop=False)
                        for jj in range(4):
                            j = c * 4 + jj
                            nc.tensor.matmul(
                                ppre_ps[:, j * 128:(j + 1) * 128],
                                peT[:, g * 1024 + j * 128:g * 1024 + (j + 1) * 128],
                                W(f'Wp2c{i}'), start=False, stop=(jj == 3),
                                skip_group_check=True)
                    ppre = sloop.tile([128, 1024], F32R, tag="big")
                    nc.scalar.activation(ppre, ppre_ps, AF.Tanh)

                    # hp0T feature-major
                    hp_ps = pbig.tile([128, 1024], F32, tag="pb")
                    for c in range(2):
                        sl = slice(c * 512, (c + 1) * 512)
                        nc.tensor.matmul(
                            hp_ps[:, sl], W(f'Whp0{i}'),
                            peT[:, g * 1024 + c * 512:g * 1024 + (c + 1) * 512],
                            start=True, stop=True)
                    hp0T = sloop.tile([128, 1024], F32, tag="big")
                    nc.scalar.activation(hp0T, hp_ps, AF.Tanh, bias=bias(f'bhp0{i}'))

                    # c_to_p (feature-major [128, 1024])
                    c2p_ps = pbig.tile([128, 1024], F32, tag="pb")
                    for c in range(2):
                        sl = slice(c * 512, (c + 1) * 512)
                        nc.tensor.matmul(
                            c2p_ps[:, sl],
                            cpn[half:half + 64, :],
                            pw2[half:half + 64,
                                k * 1024 + c * 512:k * 1024 + (c + 1) * 512],
                            start=True, stop=True)

                    # q'_p = (hp0T * wallp_g) * c2p   (f32r, feeds score matmul)
                    qp = sloop.tile([128, 1024], F32R, tag="big")
                    nc.vector.scalar_tensor_tensor(
                        out=qp, in0=hp0T, scalar=wallp[:, g:g + 1], in1=c2p_ps,
                        op0=OP.mult, op1=OP.mult)
                    if DEBUG and i == 0 and g == 0:
                        nc.sync.dma_start(out=dbg["d_hp0T_g0"], in_=hp0T)
                        nc.sync.dma_start(out=dbg["d_ppre_g0"], in_=ppre.bitcast(F32))
                        nc.sync.dma_start(out=dbg["d_qp_g0"], in_=qp.bitcast(F32))
                        c2psb = sloop.tile([128, 1024], F32, tag="big")
                        nc.vector.tensor_copy(c2psb, c2p_ps)
                        nc.sync.dma_start(out=dbg["d_c2p_g0"], in_=c2psb)

                    # scores: S_p[g, :] += sum_h qp[h, :]
                    for c in range(2):
                        sl = slice(c * 512, (c + 1) * 512)
                        nc.tensor.matmul(S_p[:, sl], OnesBlk[:, g, :], qp[:, sl],
                                         start=(g == 0), stop=(g == G - 1))

                    # p_to_c (feature-major [128, 64], accumulate 8 chunks)
                    p2c_ps = psmall.tile([128, 64], F32, tag="ps128")
                    for j in range(8):
                        nc.tensor.matmul(
                            p2c_ps,
                            ppre[:, j * 128:(j + 1) * 128],
                            pwT[:, g * 512 + j * 64:g * 512 + (j + 1) * 64],
                            start=(j == 0), stop=(j == 7))

                    if DEBUG and i == 0 and g == 0:
                        p2csb = sc64.tile([128, 64], F32, tag="qc")
                        nc.vector.tensor_copy(p2csb, p2c_ps)
                        nc.sync.dma_start(out=dbg["d_p2c_g0"], in_=p2csb)
                    qc = sc64.tile([128, 64], F32R, tag="qc")
                    nc.vector.scalar_tensor_tensor(
                        out=qc, in0=hc0T[:, g * 64:(g + 1) * 64],
                        scalar=wallc[:, g:g + 1], in1=p2c_ps,
                        op0=OP.mult, op1=OP.mult)
                    nc.tensor.matmul(S_c, OnesBlk[:, g, :], qc,
                                     start=(g == 0), stop=(g == G - 1))

                # ---- softmax stats + exp (no max subtraction: |s| stays small)
                ep_sb = sep.tile([16, 1024], F32, tag="ep")
                Zp = ssm.tile([16, 1], F32, tag="Zp")
                nc.scalar.activation(ep_sb, S_p, AF.Exp, bias=bias16(f'bhp1{i}'),
                                     accum_out=Zp)
                ec_sb = sc64.tile([16, 64], F32, tag="ec")
                Zc = ssm.tile([16, 1], F32, tag="Zc")
                nc.scalar.activation(ec_sb, S_c, AF.Exp, bias=bias16(f'bhc1{i}'),
                                     accum_out=Zc)
                nc.sync.dma_start(out=ep_scr, in_=ep_sb)
                nc.sync.dma_start(out=ec_scr, in_=ec_sb)
                if DEBUG and i == 0:
                    nc.sync.dma_start(out=dbg["d_ep0"], in_=ep_sb)
                    nc.sync.dma_start(out=dbg["d_ec0"], in_=ec_sb)
                    nc.sync.dma_start(out=dbg["d_wallp0"], in_=wallp)

                # 1/((Zc+1e-6)(Zp+1e-6)) -> row [1, G] via DRAM round trip
                Zc2 = ssm.tile([16, 1], F32, tag="Zc2")
                nc.vector.tensor_scalar(out=Zc2, in0=Zc, scalar1=1e-6, scalar2=None,
                                        op0=OP.add)
                Zcp = ssm.tile([16, 1], F32, tag="Zcp")
                nc.vector.scalar_tensor_tensor(out=Zcp, in0=Zp, scalar=1e-6,
                                               in1=Zc2, op0=OP.add, op1=OP.mult)
                rz = ssm.tile([16, 1], F32, tag="rz")
                nc.vector.reciprocal(rz, Zcp)
                nc.sync.dma_start(out=rz_scr.transpose([1, 0]), in_=rz)
                nc.sync.dma_start(out=rzrow, in_=rz_scr)

                # ---- attention pooling: cf/pf (unnormalized)
                for g in range(G):
                    ebc = sloop.tile([128, 1024], F32, tag="big")
                    nc.sync.dma_start(out=ebc,
                                      in_=ep_scr[g:g + 1, :].partition_broadcast(128))
                    nc.vector.scalar_tensor_tensor(
                        out=dump, in0=peT[:, g * 1024:(g + 1) * 1024].bitcast(F32),
                        scalar=1.0, in1=ebc, op0=OP.mult, op1=OP.mult,
                        accum_out=pfT[:, g:g + 1])
                    ecb = sc64.tile([128, 64], F32, tag="ecb")
                    nc.sync.dma_start(out=ecb,
                                      in_=ec_scr[g:g + 1, :].partition_broadcast(128))
                    nc.vector.scalar_tensor_tensor(
                        out=dumpc, in0=ceT[:, g * 64:(g + 1) * 64].bitcast(F32),
                        scalar=1.0, in1=ecb, op0=OP.mult, op1=OP.mult,
                        accum_out=cfT[:, g:g + 1])

                # ---- h = (cf * pf) / ((Zc+1e-6)(Zp+1e-6))
                hraw = ssm.tile([128, G], F32, tag="hraw")
                nc.vector.tensor_tensor(out=hraw, in0=cfT, in1=pfT, op=OP.mult)
                rzb_ps = psmall.tile([128, G], F32, tag="ps128")
                nc.tensor.matmul(rzb_ps, ones_row, rzrow, start=True, stop=True)
                hT = ssm.tile([128, G], F32R, tag="hT")
                nc.vector.tensor_tensor(out=hT, in0=hraw, in1=rzb_ps, op=OP.mult)

                # ---- GRU: m = gru(x=m, h)
                r_ps = psmall.tile([128, G], F32, tag="ps128")
                nc.tensor.matmul(r_ps, Wih(0), mT, start=True, stop=False)
                nc.tensor.matmul(r_ps, Whh(0), hT, start=False, stop=True)
                rt = ssm.tile([128, G], F32, tag="rt")
                nc.scalar.activation(rt, r_ps, AF.Tanh, bias=bias('bh_r'), scale=0.5)
                z_ps = psmall.tile([128, G], F32, tag="ps128")
                nc.tensor.matmul(z_ps, Wih(1), mT, start=True, stop=False)
                nc.tensor.matmul(z_ps, Whh(1), hT, start=False, stop=True)
                zt = ssm.tile([128, G], F32, tag="zt")
                nc.scalar.activation(zt, z_ps, AF.Tanh, bias=bias('bh_z'), scale=0.5)
                hn_ps = psmall.tile([128, G], F32, tag="ps128")
                nc.tensor.matmul(hn_ps, Whh(2), hT, start=True, stop=True)
                hn = ssm.tile([128, G], F32, tag="hn")
                nc.scalar.activation(hn, hn_ps, AF.Identity, bias=bias('bhh_n'))
                in_ps = psmall.tile([128, G], F32, tag="ps128")
                nc.tensor.matmul(in_ps, Wih(2), mT, start=True, stop=True)
                # n = tanh(inn + bih_n + 0.5*(rt*hn + hn))
                t1 = ssm.tile([128, G], F32, tag="t1")
                nc.vector.tensor_tensor(out=t1, in0=rt, in1=hn, op=OP.mult)
                t2 = ssm.tile([128, G], F32, tag="t2")
                nc.vector.tensor_tensor(out=t2, in0=t1, in1=hn, op=OP.add)
                t4 = ssm.tile([128, G], F32, tag="t4")
                nc.vector.scalar_tensor_tensor(out=t4, in0=t2, scalar=0.5,
                                               in1=in_ps, op0=OP.mult, op1=OP.add)
                nt = ssm.tile([128, G], F32, tag="nt")
                nc.scalar.activation(nt, t4, AF.Tanh, bias=bias('bih_n'))
                # m = n + 0.5*(zt*(h-n) + (h-n))
                d = ssm.tile([128, G], F32, tag="d")
                nc.vector.tensor_tensor(out=d, in0=hT, in1=nt, op=OP.subtract)
                e = ssm.tile([128, G], F32, tag="e")
                nc.vector.tensor_tensor(out=e, in0=zt, in1=d, op=OP.mult)
                f = ssm.tile([128, G], F32, tag="f")
                nc.vector.tensor_tensor(out=f, in0=e, in1=d, op=OP.add)
                f2 = ssm.tile([128, G], F32, tag="f2")
                nc.vector.tensor_scalar(out=f2, in0=f, scalar1=0.5, scalar2=None,
                                        op0=OP.mult)
                nc.vector.tensor_tensor(out=mT, in0=nt, in1=f2, op=OP.add)
                if DEBUG and i == 0:
                    nc.sync.dma_start(out=dbg["d_cf0"], in_=cfT)
                    nc.sync.dma_start(out=dbg["d_pf0"], in_=pfT)
                    nc.sync.dma_start(out=dbg["d_m1"], in_=mT.bitcast(F32))
                    nc.sync.dma_start(out=dbg["d_rz0"], in_=rzrow)

        # ================= FINAL: kron + readout =================
        with ExitStack() as fin:
            sfin = fin.enter_context(tc.tile_pool(name="sfin", bufs=2))
            sfin1 = fin.enter_context(tc.tile_pool(name="sfin1", bufs=1))
            racc = sfin1.tile([128, G], F32, tag="racc")
            nc.sync.dma_start(out=cf_scr.transpose([1, 0]), in_=cfT)
            nc.sync.dma_start(out=pf_scr.transpose([1, 0]), in_=pfT)
            cf_row = sfin1.tile([1, G * 128], F32, tag="cfr")
            pf_row = sfin1.tile([1, G * 128], F32, tag="pfr")
            nc.sync.dma_start(out=cf_row, in_=cf_scr.flatten().unsqueeze(0))
            nc.sync.dma_start(out=pf_row, in_=pf_scr.flatten().unsqueeze(0))
            for g in range(G):
                o_ps = psmall.tile([128, 128], F32, tag="ps128")
                nc.tensor.matmul(o_ps, cf_row[0:1, g * 128:(g + 1) * 128],
                                 pf_row[0:1, g * 128:(g + 1) * 128],
                                 start=True, stop=True)
                F_t = sfin.tile([128, 128], F32, tag="F")
                nc.scalar.activation(F_t, o_ps, AF.Prelu, alpha=0.1)
                G_t = sfin.tile([128, 128], F32, tag="Gt")
                nc.vector.tensor_tensor(out=G_t, in0=F_t, in1=WoS, op=OP.mult)
                nc.vector.tensor_reduce(racc[:, g:g + 1], G_t, axis=AX.X, op=OP.add)
            r_ps = psmall.tile([1, G], F32, tag="ps128")
            nc.tensor.matmul(r_ps, ones_col, racc, start=True, stop=True)
            r1 = sfin.tile([1, G], F32, tag="r1")
            nc.scalar.activation(r1, r_ps, AF.Identity)
            r2 = sfin.tile([1, G], F32, tag="r2")
            nc.vector.tensor_tensor(out=r2, in0=r1, in1=rzrow, op=OP.mult)
            r3 = sfin.tile([1, G], F32, tag="r3")
            nc.vector.tensor_scalar(out=r3, in0=r2,
                                    scalar1=BfS[0:1, BC['b_out']:BC['b_out'] + 1],
                                    scalar2=None, op0=OP.add)
            nc.sync.dma_start(out=out_d, in_=r3)

    nc.compile()
    return nc, WC, BC, NW, NB


def _host_pack(params, WC, BC, NW, NB):
    p = {k: np.asarray(v, dtype=np.float32) for k, v in params.items()}
    Wr = np.zeros((128, NW), np.float32)
    for name in ['Wpc', 'Wpp', 'Wca', 'Wpa']:
        Wr[:, WC[name]:WC[name] + 128] = p[name]
    for name in ['Wc2p', 'Wp2c', 'Wmc1', 'Wmp1', 'Whc0', 'Whp0']:
        for i in range(D):
            Wr[:, WC[f'{name}{i}']:WC[f'{name}{i}'] + 128] = p[name][i]
    Wr[:, WC['WihT']:WC['WihT'] + 384] = p['Wih'].T
    Wr[:, WC['WhhT']:WC['WhhT'] + 384] = p['Whh'].T
    for i in range(D):
        Wr[0, WC[f'bc2p_row{i}']:WC[f'bc2p_row{i}'] + 128] = p['bc2p'][i]
        Wr[0, WC[f'bp2c_row{i}']:WC[f'bp2c_row{i}'] + 512] = np.tile(p['bp2c'][i], 4)

    Bf = np.zeros((128, NB), np.float32)
    for name in ['bpc', 'bpp', 'bca', 'bpa']:
        Bf[:, BC[name]] = p[name]
    for name in ['bc2p', 'bp2c', 'bmc1', 'bmp1', 'bhc0', 'bhp0']:
        for i in range(D):
            Bf[:, BC[f'{name}{i}']] = p[name][i]
    for i in range(D):
        Bf[:, BC[f'Whc1{i}']] = p['Whc1'][i][:, 0]
        Bf[:, BC[f'Whp1{i}']] = p['Whp1'][i][:, 0]
        Bf[:, BC[f'bhp1{i}']] = p['bhp1'][i][0]
        Bf[:, BC[f'bhc1{i}']] = p['bhc1'][i][0]
    Bf[:, BC['bh_r']] = 0.5 * (p['bih'][0:128] + p['bhh'][0:128])
    Bf[:, BC['bh_z']] = 0.5 * (p['bih'][128:256] + p['bhh'][128:256])
    Bf[:, BC['bih_n']] = p['bih'][256:384]
    Bf[:, BC['bhh_n']] = p['bhh'][256:384]
    Bf[:, BC['b_out']] = p['b_out'][0]

    Wo = np.ascontiguousarray(p['W_out'].reshape(H, H))
    return Wr, Bf, Wo


def kernel(comp_feature, prot_feature, batch_comp, batch_prot, params):
    from concourse.bass_utils import run_bass_kernel_spmd

    if 'nc' not in _cache:
        _cache['nc'], _cache['WC'], _cache['BC'], _cache['NW'], _cache['NB'] = _build()
    nc = _cache['nc']

    comp = np.asarray(comp_feature, dtype=np.float32)
    prot = np.asarray(prot_feature, dtype=np.float32)
    Wr, Bf, Wo = _host_pack(params, _cache['WC'], _cache['BC'],
                            _cache['NW'], _cache['NB'])

    comp3 = comp.reshape(B, LC, 128)
    prot3 = prot.reshape(B, LP, 128)
    in_maps = []
    for c in range(NCORES):
        gs = slice(c * G, (c + 1) * G)
        compT = np.ascontiguousarray(
            comp3[gs].transpose(2, 0, 1).reshape(128, G * LC))
        protT = np.ascontiguousarray(
            prot3[gs].transpose(2, 0, 1).reshape(128, G * LP))
        in_maps.append({'protT': protT, 'compT': compT,
                        'Wr': Wr, 'Bf': Bf, 'Wo': Wo})

    res = run_bass_kernel_spmd(nc, in_maps, core_ids=list(range(NCORES)))
    out = np.concatenate([res.results[c]['out'].reshape(G) for c in range(NCORES)])
    return out.reshape(B, 1).astype(np.float32)


# revision 11
# speedup vs baseline: 1.0402x; 1.0402x over previous
"""Trainium2 Bass kernel for AffinityNeuralNetworkCliffNetMONN.

Sharding: data-parallel over the B=128 graphs; each of the 8 NeuronCores
processes 16 whole graphs. Weights are replicated. Host pre-transposes node
features to feature-major layout and packs weights; device computes the full
network; host concatenates the per-core [16] affinity outputs into [B, 1].

Device layouts (per core, G=16 graphs):
  - protT [128, G*1024], compT [128, G*64]: feature-major node features (f32r)
  - activations feature-major [H=128 partitions, nodes free]
  - pw packed 2-graphs-per-128-partitions: pw2 [128, 8*1024]
  - pwT (transposed pairwise map) [128, G*512], chunk-major
  - node softmax on [16, nodes] tiles (graph per partition); exp rows are
    broadcast back to 128 partitions via a DRAM round-trip DMA.
Matmuls run in float32r (full PE rate, ~1.6e-4 matmul precision). The
reference's sigmoid is computed as 0.5*tanh(0.5x)+0.5 so the whole kernel
stays in the exp/tanh/lrelu activation-table set (no table switches).
"""
import sys

sys.path.insert(0, '/opt/trn_rl_repo')

import numpy as np

B, LC, LP, H, D = 128, 64, 1024, 128, 3
NCORES = 8
G = B // NCORES  # 16 graphs per core

_cache = {}
DEBUG = False


def _build():
    from contextlib import ExitStack
    import concourse.bass as bass
    from concourse import bacc
    import concourse.tile as tile
    from concourse import mybir
    from concourse.masks import make_identity

    F32 = mybir.dt.float32
    F32R = mybir.dt.float32r
    BF16 = mybir.dt.bfloat16
    AF = mybir.ActivationFunctionType
    OP = mybir.AluOpType
    AX = mybir.AxisListType

    # ---- bias/vector pack column map (fp32 pack, one [128,1] column each)
    BC = {}
    col = 0
    for name in ['bpc', 'bpp', 'bca', 'bpa']:
        BC[name] = col; col += 1
    for name in ['bc2p', 'bp2c', 'bmc1', 'bmp1', 'bhc0', 'bhp0',
                 'Whc1', 'Whp1', 'bhp1', 'bhc1']:
        for i in range(D):
            BC[f'{name}{i}'] = col; col += 1
    for name in ['bh_r', 'bh_z', 'bih_n', 'bhh_n', 'b_out']:
        BC[name] = col; col += 1
    NB = col

    # ---- matmul weight pack column map (f32r pack)
    WC = {}
    wcol = 0
    for name in ['Wpc', 'Wpp', 'Wca', 'Wpa']:
        WC[name] = wcol; wcol += 128
    for name in ['Wc2p', 'Wp2c', 'Wmc1', 'Wmp1', 'Whc0', 'Whp0']:
        for i in range(D):
            WC[f'{name}{i}'] = wcol; wcol += 128
    WC['WihT'] = wcol; wcol += 3 * 128
    WC['WhhT'] = wcol; wcol += 3 * 128
    for i in range(D):
        WC[f'bc2p_row{i}'] = wcol; wcol += 128   # [1,128] row bias (c_pre)
        WC[f'bp2c_row{i}'] = wcol; wcol += 512   # [1,512] = 4 replicas (p_pre)
    NW = wcol

    nc = bacc.Bacc("TRN2", target_bir_lowering=False, debug=False)

    protT_d = nc.dram_tensor("protT", (128, G * 1024), F32R, kind="ExternalInput").ap()
    compT_d = nc.dram_tensor("compT", (128, G * 64), F32R, kind="ExternalInput").ap()
    Wr_d = nc.dram_tensor("Wr", (128, NW), F32R, kind="ExternalInput").ap()
    Bf_d = nc.dram_tensor("Bf", (128, NB), F32, kind="ExternalInput").ap()
    Wo_d = nc.dram_tensor("Wo", (128, 128), F32, kind="ExternalInput").ap()
    out_d = nc.dram_tensor("out", (1, G), F32, kind="ExternalOutput").ap()
    dbg = {}
    if DEBUG:
        for nm, shp in [("d_m0", (128, G)), ("d_pw0", (64, 1024)),
                        ("d_pw1", (64, 1024)), ("d_pwT0", (128, 512)),
                        ("d_ep0", (G, 1024)), ("d_ec0", (G, 64)),
                        ("d_cf0", (128, G)), ("d_pf0", (128, G)),
                        ("d_m1", (128, G)), ("d_wallp0", (128, G)),
                        ("d_hp0T_g0", (128, 1024)), ("d_c2p_g0", (128, 1024)),
                        ("d_qp_g0", (128, 1024)), ("d_ppre_g0", (128, 1024)),
                        ("d_p2c_g0", (128, 64)), ("d_rz0", (1, G))]:
            dbg[nm] = nc.dram_tensor(nm, shp, F32, kind="ExternalOutput").ap()

    ep_scr = nc.dram_tensor("ep_scr", (G, 1024), BF16, kind="Internal").ap()
    ec_scr = nc.dram_tensor("ec_scr", (G, 64), BF16, kind="Internal").ap()
    rz_scr = nc.dram_tensor("rz_scr", (1, G), F32, kind="Internal").ap()
    cf_scr = nc.dram_tensor("cf_scr", (G, 128), F32, kind="Internal").ap()
    pf_scr = nc.dram_tensor("pf_scr", (G, 128), F32, kind="Internal").ap()

    with tile.TileContext(nc) as tc, ExitStack() as top:
        persist = top.enter_context(tc.tile_pool(name="persist", bufs=1))
        pbig = top.enter_context(tc.tile_pool(name="pbig", bufs=2, space="PSUM"))
        psmall = top.enter_context(tc.tile_pool(name="psmall", bufs=1, space="PSUM"))

        # ---------------- constants / weights ----------------
        WrS = persist.tile([128, NW], F32R)
        BfS = persist.tile([128, NB], F32)
        WoS = persist.tile([128, 128], F32)
        nc.sync.dma_start(out=WrS, in_=Wr_d)
        nc.sync.dma_start(out=BfS, in_=Bf_d)
        nc.sync.dma_start(out=WoS, in_=Wo_d)

        ident = persist.tile([128, 128], F32)
        make_identity(nc, ident)

        ones_row = persist.tile([1, 128], F32)
        nc.vector.memset(ones_row, 1.0)
        ones_col = persist.tile([128, 1], F32)
        nc.vector.memset(ones_col, 1.0)
        ones_row_r = persist.tile([1, 128], F32R)
        nc.vector.tensor_copy(ones_row_r, ones_row)


        def W(name):
            return WrS[:, WC[name]:WC[name] + 128]

        def Wih(j):
            return WrS[:, WC['WihT'] + j * 128:WC['WihT'] + (j + 1) * 128]

        def Whh(j):
            return WrS[:, WC['WhhT'] + j * 128:WC['WhhT'] + (j + 1) * 128]

        def bias(name):
            return BfS[:, BC[name]:BC[name] + 1]

        def bias16(name):
            return BfS[0:16, BC[name]:BC[name] + 1]

        # ---------------- persistent activations ----------------
        peT = persist.tile([128, G * 1024], F32R)   # lrelu prot features
        ceT = persist.tile([128, G * 64], F32R)     # lrelu comp features
        pw2 = persist.tile([128, (G // 2) * 1024], F32R)  # pairwise, 2-graph packed
        pwT = persist.tile([128, G * 512], F32R)    # pairwise transposed, chunked
        csum = persist.tile([128, G], F32)
        psums = persist.tile([128, G], F32)
        mT = persist.tile([128, G], F32R)
        cfT = persist.tile([128, G], F32)
        pfT = persist.tile([128, G], F32)
        rzrow = persist.tile([1, G], F32)
        dump = persist.tile([128, 1024], F32)
        dumpc = persist.tile([128, 64], F32)

        # ================= PREP PHASE =================
        with ExitStack() as prep:
            sprep = prep.enter_context(tc.tile_pool(name="sprep", bufs=1))
            strm = prep.enter_context(tc.tile_pool(name="strm", bufs=2))
            pprep = prep.enter_context(tc.tile_pool(name="pprep", bufs=1, space="PSUM"))

            compT = sprep.tile([128, G * 64], F32R, tag="compT")
            nc.sync.dma_start(out=compT, in_=compT_d)

            # pcT / ceT for all graphs  (2x N=512 matmuls each)
            pcT = sprep.tile([128, G * 64], F32R, tag="pcT")
            pc_ps = pbig.tile([128, 1024], F32, tag="pb")
            ce_ps = pbig.tile([128, 1024], F32, tag="pb")
            for c in range(2):
                sl = slice(c * 512, (c + 1) * 512)
                nc.tensor.matmul(pc_ps[:, sl], W('Wpc'), compT[:, sl],
                                 start=True, stop=True)
                nc.tensor.matmul(ce_ps[:, sl], W('Wca'), compT[:, sl],
                                 start=True, stop=True)
            nc.scalar.activation(pcT, pc_ps, AF.Prelu, bias=bias('bpc'), alpha=0.1)
            for g in range(G):
                sl = slice(g * 64, (g + 1) * 64)
                nc.scalar.activation(ceT[:, sl], ce_ps[:, sl], AF.Prelu,
                                     bias=bias('bca'), alpha=0.1,
                                     accum_out=csum[:, g:g + 1])

            pwps = None
            for g in range(G):
                psl = slice(g * 1024, (g + 1) * 1024)
                protT = strm.tile([128, 1024], F32R, tag="protT")
                nc.sync.dma_start(out=protT, in_=protT_d[:, psl])

                pp_ps = pbig.tile([128, 1024], F32, tag="pb")
                pe_ps = pbig.tile([128, 1024], F32, tag="pb")
                for c in range(2):
                    sl = slice(c * 512, (c + 1) * 512)
                    nc.tensor.matmul(pp_ps[:, sl], W('Wpp'), protT[:, sl],
                                     start=True, stop=True)
                    nc.tensor.matmul(pe_ps[:, sl], W('Wpa'), protT[:, sl],
                                     start=True, stop=True)
                ppT = strm.tile([128, 1024], F32R, tag="ppT")
                nc.scalar.activation(ppT, pp_ps, AF.Prelu, bias=bias('bpp'), alpha=0.1)
                nc.scalar.activation(peT[:, psl], pe_ps, AF.Prelu, bias=bias('bpa'),
                                     alpha=0.1, accum_out=psums[:, g:g + 1])

                # pairwise map z = pcT_g.T @ ppT  ->  rows (g%2)*64 of pair psum
                half = (g % 2) * 64
                if g % 2 == 0:
                    pwps = pprep.tile([128, 1024], F32, tag="pwps")
                for c in range(2):
                    sl = slice(c * 512, (c + 1) * 512)
                    # f32r matmuls cannot target dst partition 64: use fp32
                    # there (operand values are already f32r-rounded).
                    if half:
                        nc.tensor.matmul(pwps[half:half + 64, sl],
                                         pcT[:, g * 64:(g + 1) * 64].bitcast(F32),
                                         ppT[:, sl].bitcast(F32),
                                         start=True, stop=True)
                    else:
                        nc.tensor.matmul(pwps[half:half + 64, sl],
                                         pcT[:, g * 64:(g + 1) * 64], ppT[:, sl],
                                         start=True, stop=True)
                if g % 2 == 1:
                    k = g // 2
                    ksl = slice(k * 1024, (k + 1) * 1024)
                    # sigmoid(z) = 0.5*tanh(0.5 z) + 0.5
                    tsig = strm.tile([128, 1024], F32, tag="tsig")
                    nc.scalar.activation(tsig, pwps, AF.Tanh, scale=0.5)
                    nc.vector.tensor_scalar(out=pw2[:, ksl], in0=tsig,
                                            scalar1=0.5, scalar2=0.5,
                                            op0=OP.mult, op1=OP.add)

            # transpose pw -> pwT   (per graph: 8 PE transposes of [64,128])
            for g in range(G):
                half = (g % 2) * 64
                k = g // 2
                trp = pprep.tile([128, 8, 64], F32, tag="trp")
                for j in range(8):
                    src = pw2[half:half + 64,
                              k * 1024 + j * 128:k * 1024 + (j + 1) * 128]
                    nc.tensor.transpose(
                        trp[:, j], src.bitcast(F32),
                        ident[half:half + 64, half:half + 64])
                nc.vector.tensor_copy(
                    pwT[:, g * 512:(g + 1) * 512],
                    trp.rearrange("p a b -> p (a b)"))

            # m0 = mean(ce) * mean(pe)
            m0t = sprep.tile([128, G], F32, tag="m0t")
            nc.vector.tensor_tensor(out=m0t, in0=csum, in1=psums, op=OP.mult)
            nc.vector.tensor_scalar(out=mT, in0=m0t, scalar1=1.0 / (64.0 * 1024.0),
                                    scalar2=None, op0=OP.mult)
            if DEBUG:
                nc.sync.dma_start(out=dbg["d_m0"], in_=mT.bitcast(F32))
                nc.sync.dma_start(out=dbg["d_pw0"], in_=pw2[0:64, 0:1024].bitcast(F32))
                nc.sync.dma_start(out=dbg["d_pw1"], in_=pw2[64:128, 0:1024].bitcast(F32))
                nc.sync.dma_start(out=dbg["d_pwT0"], in_=pwT[:, 0:512].bitcast(F32))

        # ================= MESSAGE-PASSING LOOP =================
        with ExitStack() as loop:
            sloop = loop.enter_context(tc.tile_pool(name="sloop", bufs=4))
            swb = loop.enter_context(tc.tile_pool(name="swb", bufs=2))
            sep = loop.enter_context(tc.tile_pool(name="sep", bufs=2))
            sebc = loop.enter_context(tc.tile_pool(name="sebc", bufs=3))
            sc64 = loop.enter_context(tc.tile_pool(name="sc64", bufs=4))
            spair = loop.enter_context(tc.tile_pool(name="spair", bufs=4))
            ssm = loop.enter_context(tc.tile_pool(name="ssm", bufs=4))
            shc = loop.enter_context(tc.tile_pool(name="shc", bufs=1))
            pS = loop.enter_context(tc.tile_pool(name="pS", bufs=1, space="PSUM"))

            for i in range(D):
                # ---- m-dependent folded score weights
                mc_ps = psmall.tile([128, G], F32, tag="ps128")
                nc.tensor.matmul(mc_ps, W(f'Wmc1{i}'), mT, start=True, stop=True)
                mcT = ssm.tile([128, G], F32, tag="mcT")
                nc.scalar.activation(mcT, mc_ps, AF.Tanh, bias=bias(f'bmc1{i}'))
                wallc = ssm.tile([128, G], F32, tag="wallc")
                nc.vector.tensor_scalar(out=wallc, in0=mcT, scalar1=bias(f'Whc1{i}'),
                                        scalar2=None, op0=OP.mult)
                wblkCF = swb.tile([128, G, G], F32, tag="stage")
                nc.vector.memset(wblkCF, 0.0)
                diagC = bass.AP(tensor=wblkCF.tensor, offset=wblkCF.offset,
                                ap=[wblkCF.ap[0], [G + 1, G]])
                nc.vector.tensor_copy(diagC, wallc)
                wblkC = swb.tile([128, G, G], F32R, tag="wblkC")
                nc.vector.tensor_copy(wblkC, wblkCF)

                mp_ps = psmall.tile([128, G], F32, tag="ps128")
                nc.tensor.matmul(mp_ps, W(f'Wmp1{i}'), mT, start=True, stop=True)
                mpT = ssm.tile([128, G], F32, tag="mpT")
                nc.scalar.activation(mpT, mp_ps, AF.Tanh, bias=bias(f'bmp1{i}'))
                wallp = ssm.tile([128, G], F32, tag="wallp")
                nc.vector.tensor_scalar(out=wallp, in0=mpT, scalar1=bias(f'Whp1{i}'),
                                        scalar2=None, op0=OP.mult)
                wblkPF = swb.tile([128, G, G], F32, tag="stage")
                nc.vector.memset(wblkPF, 0.0)
                diagP = bass.AP(tensor=wblkPF.tensor, offset=wblkPF.offset,
                                ap=[wblkPF.ap[0], [G + 1, G]])
                nc.vector.tensor_copy(diagP, wallp)
                wblkP = swb.tile([128, G, G], F32R, tag="wblkP")
                nc.vector.tensor_copy(wblkP, wblkPF)

                # ---- hc0T for all graphs (feature-major)
                hc_ps = pbig.tile([128, 1024], F32, tag="pb")
                for c in range(2):
                    sl = slice(c * 512, (c + 1) * 512)
                    nc.tensor.matmul(hc_ps[:, sl], W(f'Whc0{i}'), ceT[:, sl],
                                     start=True, stop=True)
                hc0T = shc.tile([128, G * 64], F32, tag="hc0T")
                nc.scalar.activation(hc0T, hc_ps, AF.Tanh, bias=bias(f'bhc0{i}'))

                S_p = pS.tile([16, 1024], F32, tag="Sp")
                S_c = pS.tile([16, 64], F32, tag="Sc")

                cpn = None
                for g in range(G):
                    half = (g % 2) * 64
                    k = g // 2

                    # ---- c_pre natural, 2-graph packed [128(c), 128(h)]
                    if g % 2 == 0:
                        cp_ps = psmall.tile([128, 128], F32, tag="ps128")
                        # natural layout [node, h]: bias lives on the free dim;
                        # seed psum with ones x bias_row, then accumulate.
                        bc_row = WrS[0:1, WC[f'bc2p_row{i}']:WC[f'bc2p_row{i}'] + 128]
                        nc.tensor.matmul(cp_ps, ones_row_r, bc_row,
                                         start=True, stop=False)
                        for hh in (0, 64):
                            gg = 2 * k + (hh // 64)
                            if hh:
                                nc.tensor.matmul(
                                    cp_ps[hh:hh + 64, :],
                                    ceT[:, gg * 64:(gg + 1) * 64].bitcast(F32),
                                    W(f'Wc2p{i}').bitcast(F32),
                                    start=False, stop=(hh == 64),
                                    skip_group_check=True)
                            else:
                                nc.tensor.matmul(cp_ps[hh:hh + 64, :],
                                                 ceT[:, gg * 64:(gg + 1) * 64],
                                                 W(f'Wc2p{i}'), start=False,
                                                 stop=False, skip_group_check=True)
                        cpn = spair.tile([128, 128], F32R, tag="cpn")
                        nc.scalar.activation(cpn, cp_ps, AF.Tanh)

                    # p_pre natural: 8 chunk matmuls peT_chunk.T @ Wp2c
                    ppre_ps = pbig.tile([128, 1024], F32, tag="pb")
                    bp_row = WrS[0:1, WC[f'bp2c_row{i}']:WC[f'bp2c_row{i}'] + 512]
                    for c in range(2):
                        nc.tensor.matmul(ppre_ps[:, c * 512:(c + 1) * 512],
                                         ones_row_r, bp_row, start=True, stop=False)
                        for jj in range(4):
                            j = c * 4 + jj
                            nc.tensor.matmul(
                                ppre_ps[:, j * 128:(j + 1) * 128],
                                peT[:, g * 1024 + j * 128:g * 1024 + (j + 1) * 128],
                                W(f'Wp2c{i}'), start=False, stop=(jj == 3),
                                skip_group_check=True)
                    ppre = sloop.tile([128, 1024], F32R, tag="big")
                    nc.scalar.activation(ppre, ppre_ps, AF.Tanh)

                    # hp0T feature-major
                    hp_ps = pbig.tile([128, 1024], F32, tag="pb")
                    for c in range(2):
                        sl = slice(c * 512, (c + 1) * 512)
                        nc.tensor.matmul(
                            hp_ps[:, sl], W(f'Whp0{i}'),
                            peT[:, g * 1024 + c * 512:g * 1024 + (c + 1) * 512],
                            start=True, stop=True)
                    hp0T = sloop.tile([128, 1024], F32, tag="big")
                    nc.scalar.activation(hp0T, hp_ps, AF.Tanh, bias=bias(f'bhp0{i}'))

                    # c_to_p (feature-major [128, 1024])
                    c2p_ps = pbig.tile([128, 1024], F32, tag="pb")
                    for c in range(2):
                        sl = slice(c * 512, (c + 1) * 512)
                        nc.tensor.matmul(
                            c2p_ps[:, sl],
                            cpn[half:half + 64, :],
                            pw2[half:half + 64,
                                k * 1024 + c * 512:k * 1024 + (c + 1) * 512],
                            start=True, stop=True)

                    # u_p = hp0T * c2p  (m-independent; w folded into score lhsT)
                    qp = sloop.tile([128, 1024], F32R, tag="big")
                    nc.vector.tensor_tensor(out=qp, in0=hp0T, in1=c2p_ps, op=OP.mult)
                    if DEBUG and i == 0 and g == 0:
                        nc.sync.dma_start(out=dbg["d_hp0T_g0"], in_=hp0T)
                        nc.sync.dma_start(out=dbg["d_ppre_g0"], in_=ppre.bitcast(F32))
                        nc.sync.dma_start(out=dbg["d_qp_g0"], in_=qp.bitcast(F32))
                        c2psb = sloop.tile([128, 1024], F32, tag="big")
                        nc.vector.tensor_copy(c2psb, c2p_ps)
                        nc.sync.dma_start(out=dbg["d_c2p_g0"], in_=c2psb)

                    # scores: S_p[g, :] += sum_h qp[h, :]
                    for c in range(2):
                        sl = slice(c * 512, (c + 1) * 512)
                        nc.tensor.matmul(S_p[:, sl], wblkP[:, g, :], qp[:, sl],
                                         start=(g == 0), stop=(g == G - 1))

                    # p_to_c (feature-major [128, 64], accumulate 8 chunks)
                    p2c_ps = psmall.tile([128, 64], F32, tag="ps128")
                    for j in range(8):
                        nc.tensor.matmul(
                            p2c_ps,
                            ppre[:, j * 128:(j + 1) * 128],
                            pwT[:, g * 512 + j * 64:g * 512 + (j + 1) * 64],
                            start=(j == 0), stop=(j == 7))

                    if DEBUG and i == 0 and g == 0:
                        p2csb = sc64.tile([128, 64], F32, tag="qc")
                        nc.vector.tensor_copy(p2csb, p2c_ps)
                        nc.sync.dma_start(out=dbg["d_p2c_g0"], in_=p2csb)
                    qc = sc64.tile([128, 64], F32R, tag="qc")
                    nc.vector.tensor_tensor(out=qc, in0=hc0T[:, g * 64:(g + 1) * 64],
                                            in1=p2c_ps, op=OP.mult)
                    nc.tensor.matmul(S_c, wblkC[:, g, :], qc,
                                     start=(g == 0), stop=(g == G - 1))

                # ---- softmax stats + exp (no max subtraction: |s| stays small)
                ep_sb = sep.tile([16, 1024], BF16, tag="ep")
                Zp = ssm.tile([16, 1], F32, tag="Zp")
                nc.scalar.activation(ep_sb, S_p, AF.Exp, bias=bias16(f'bhp1{i}'),
                                     accum_out=Zp)
                ec_sb = sc64.tile([16, 64], BF16, tag="ec")
                Zc = ssm.tile([16, 1], F32, tag="Zc")
                nc.scalar.activation(ec_sb, S_c, AF.Exp, bias=bias16(f'bhc1{i}'),
                                     accum_out=Zc)
                nc.sync.dma_start(out=ep_scr, in_=ep_sb)
                nc.sync.dma_start(out=ec_scr, in_=ec_sb)
                if DEBUG and i == 0:
                    nc.sync.dma_start(out=dbg["d_ep0"], in_=ep_sb)
                    nc.sync.dma_start(out=dbg["d_ec0"], in_=ec_sb)
                    nc.sync.dma_start(out=dbg["d_wallp0"], in_=wallp)

                # 1/((Zc+1e-6)(Zp+1e-6)) -> row [1, G] via DRAM round trip
                Zc2 = ssm.tile([16, 1], F32, tag="Zc2")
                nc.vector.tensor_scalar(out=Zc2, in0=Zc, scalar1=1e-6, scalar2=None,
                                        op0=OP.add)
                Zcp = ssm.tile([16, 1], F32, tag="Zcp")
                nc.vector.scalar_tensor_tensor(out=Zcp, in0=Zp, scalar=1e-6,
                                               in1=Zc2, op0=OP.add, op1=OP.mult)
                rz = ssm.tile([16, 1], F32, tag="rz")
                nc.vector.reciprocal(rz, Zcp)
                nc.sync.dma_start(out=rz_scr.transpose([1, 0]), in_=rz)
                nc.sync.dma_start(out=rzrow, in_=rz_scr)

                # ---- attention pooling: cf/pf (unnormalized)
                for g in range(G):
                    ebc = sebc.tile([128, 1024], BF16, tag="ebc")
                    nc.sync.dma_start(out=ebc,
                                      in_=ep_scr[g:g + 1, :].partition_broadcast(128))
                    nc.vector.scalar_tensor_tensor(
                        out=dump, in0=peT[:, g * 1024:(g + 1) * 1024].bitcast(F32),
                        scalar=1.0, in1=ebc, op0=OP.mult, op1=OP.mult,
                        accum_out=pfT[:, g:g + 1])
                    ecb = sebc.tile([128, 64], BF16, tag="ecb")
                    nc.sync.dma_start(out=ecb,
                                      in_=ec_scr[g:g + 1, :].partition_broadcast(128))
                    nc.vector.scalar_tensor_tensor(
                        out=dumpc, in0=ceT[:, g * 64:(g + 1) * 64].bitcast(F32),
                        scalar=1.0, in1=ecb, op0=OP.mult, op1=OP.mult,
                        accum_out=cfT[:, g:g + 1])

                # ---- h = (cf * pf) / ((Zc+1e-6)(Zp+1e-6))
                hraw = ssm.tile([128, G], F32, tag="hraw")
                nc.vector.tensor_tensor(out=hraw, in0=cfT, in1=pfT, op=OP.mult)
                rzb_ps = psmall.tile([128, G], F32, tag="ps128")
                nc.tensor.matmul(rzb_ps, ones_row, rzrow, start=True, stop=True)
                hT = ssm.tile([128, G], F32R, tag="hT")
                nc.vector.tensor_tensor(out=hT, in0=hraw, in1=rzb_ps, op=OP.mult)

                # ---- GRU: m = gru(x=m, h)
                r_ps = psmall.tile([128, G], F32, tag="ps128")
                nc.tensor.matmul(r_ps, Wih(0), mT, start=True, stop=False)
                nc.tensor.matmul(r_ps, Whh(0), hT, start=False, stop=True)
                rt = ssm.tile([128, G], F32, tag="rt")
                nc.scalar.activation(rt, r_ps, AF.Tanh, bias=bias('bh_r'), scale=0.5)
                z_ps = psmall.tile([128, G], F32, tag="ps128")
                nc.tensor.matmul(z_ps, Wih(1), mT, start=True, stop=False)
                nc.tensor.matmul(z_ps, Whh(1), hT, start=False, stop=True)
                zt = ssm.tile([128, G], F32, tag="zt")
                nc.scalar.activation(zt, z_ps, AF.Tanh, bias=bias('bh_z'), scale=0.5)
                hn_ps = psmall.tile([128, G], F32, tag="ps128")
                nc.tensor.matmul(hn_ps, Whh(2), hT, start=True, stop=True)
                hn = ssm.tile([128, G], F32, tag="hn")
                nc.scalar.activation(hn, hn_ps, AF.Identity, bias=bias('bhh_n'))
                in_ps = psmall.tile([128, G], F32, tag="ps128")
                nc.tensor.matmul(in_ps, Wih(2), mT, start=True, stop=True)
                # n = tanh(inn + bih_n + 0.5*(rt*hn + hn))
                t1 = ssm.tile([128, G], F32, tag="t1")
                nc.vector.tensor_tensor(out=t1, in0=rt, in1=hn, op=OP.mult)
                t2 = ssm.tile([128, G], F32, tag="t2")
                nc.vector.tensor_tensor(out=t2, in0=t1, in1=hn, op=OP.add)
                t4 = ssm.tile([128, G], F32, tag="t4")
                nc.vector.scalar_tensor_tensor(out=t4, in0=t2, scalar=0.5,
                                               in1=in_ps, op0=OP.mult, op1=OP.add)
                nt = ssm.tile([128, G], F32, tag="nt")
                nc.scalar.activation(nt, t4, AF.Tanh, bias=bias('bih_n'))
                # m = n + 0.5*(zt*(h-n) + (h-n))
                d = ssm.tile([128, G], F32, tag="d")
                nc.vector.tensor_tensor(out=d, in0=hT, in1=nt, op=OP.subtract)
                e = ssm.tile([128, G], F32, tag="e")
                nc.vector.tensor_tensor(out=e, in0=zt, in1=d, op=OP.mult)
                f = ssm.tile([128, G], F32, tag="f")
                nc.vector.tensor_tensor(out=f, in0=e, in1=d, op=OP.add)
                f2 = ssm.tile([128, G], F32, tag="f2")
                nc.vector.tensor_scalar(out=f2, in0=f, scalar1=0.5, scalar2=None,
                                        op0=OP.mult)
                nc.vector.tensor_tensor(out=mT, in0=nt, in1=f2, op=OP.add)
                if DEBUG and i == 0:
                    nc.sync.dma_start(out=dbg["d_cf0"], in_=cfT)
                    nc.sync.dma_start(out=dbg["d_pf0"], in_=pfT)
                    nc.sync.dma_start(out=dbg["d_m1"], in_=mT.bitcast(F32))
                    nc.sync.dma_start(out=dbg["d_rz0"], in_=rzrow)

        # ================= FINAL: kron + readout =================
        with ExitStack() as fin:
            sfin = fin.enter_context(tc.tile_pool(name="sfin", bufs=2))
            sfin1 = fin.enter_context(tc.tile_pool(name="sfin1", bufs=1))
            racc = sfin1.tile([128, G], F32, tag="racc")
            cft_ps = pbig.tile([128, 1024], F32, tag="pb")
            nc.tensor.transpose(cft_ps[0:16, 0:128], cfT, ident)
            cfrow_sb = sfin1.tile([16, 128], F32, tag="cfrs")
            nc.vector.tensor_copy(cfrow_sb, cft_ps[0:16, 0:128])
            nc.sync.dma_start(out=cf_scr, in_=cfrow_sb)
            pft_ps = pbig.tile([128, 1024], F32, tag="pb")
            nc.tensor.transpose(pft_ps[0:16, 0:128], pfT, ident)
            pfrow_sb = sfin1.tile([16, 128], F32, tag="pfrs")
            nc.vector.tensor_copy(pfrow_sb, pft_ps[0:16, 0:128])
            nc.sync.dma_start(out=pf_scr, in_=pfrow_sb)
            cf_row = sfin1.tile([1, G * 128], F32, tag="cfr")
            pf_row = sfin1.tile([1, G * 128], F32, tag="pfr")
            nc.sync.dma_start(out=cf_row, in_=cf_scr.flatten().unsqueeze(0))
            nc.sync.dma_start(out=pf_row, in_=pf_scr.flatten().unsqueeze(0))
            for g in range(G):
                o_ps = psmall.tile([128, 128], F32, tag="ps128")
                nc.tensor.matmul(o_ps, cf_row[0:1, g * 128:(g + 1) * 128],
                                 pf_row[0:1, g * 128:(g + 1) * 128],
                                 start=True, stop=True)
                F_t = sfin.tile([128, 128], F32, tag="F")
                nc.scalar.activation(F_t, o_ps, AF.Prelu, alpha=0.1)
                G_t = sfin.tile([128, 128], F32, tag="Gt")
                nc.vector.tensor_tensor(out=G_t, in0=F_t, in1=WoS, op=OP.mult)
                nc.vector.tensor_reduce(racc[:, g:g + 1], G_t, axis=AX.X, op=OP.add)
            r_ps = psmall.tile([1, G], F32, tag="ps128")
            nc.tensor.matmul(r_ps, ones_col, racc, start=True, stop=True)
            r1 = sfin.tile([1, G], F32, tag="r1")
            nc.scalar.activation(r1, r_ps, AF.Identity)
            r2 = sfin.tile([1, G], F32, tag="r2")
            nc.vector.tensor_tensor(out=r2, in0=r1, in1=rzrow, op=OP.mult)
            r3 = sfin.tile([1, G], F32, tag="r3")
            nc.vector.tensor_scalar(out=r3, in0=r2,
                                    scalar1=BfS[0:1, BC['b_out']:BC['b_out'] + 1],
                                    scalar2=None, op0=OP.add)
            nc.sync.dma_start(out=out_d, in_=r3)

    nc.compile()
    return nc, WC, BC, NW, NB


def _host_pack(params, WC, BC, NW, NB):
    p = {k: np.asarray(v, dtype=np.float32) for k, v in params.items()}
    Wr = np.zeros((128, NW), np.float32)
    for name in ['Wpc', 'Wpp', 'Wca', 'Wpa']:
        Wr[:, WC[name]:WC[name] + 128] = p[name]
    for name in ['Wc2p', 'Wp2c', 'Wmc1', 'Wmp1', 'Whc0', 'Whp0']:
        for i in range(D):
            Wr[:, WC[f'{name}{i}']:WC[f'{name}{i}'] + 128] = p[name][i]
    Wr[:, WC['WihT']:WC['WihT'] + 384] = p['Wih'].T
    Wr[:, WC['WhhT']:WC['WhhT'] + 384] = p['Whh'].T
    for i in range(D):
        Wr[0, WC[f'bc2p_row{i}']:WC[f'bc2p_row{i}'] + 128] = p['bc2p'][i]
        Wr[0, WC[f'bp2c_row{i}']:WC[f'bp2c_row{i}'] + 512] = np.tile(p['bp2c'][i], 4)

    Bf = np.zeros((128, NB), np.float32)
    for name in ['bpc', 'bpp', 'bca', 'bpa']:
        Bf[:, BC[name]] = p[name]
    for name in ['bc2p', 'bp2c', 'bmc1', 'bmp1', 'bhc0', 'bhp0']:
        for i in range(D):
            Bf[:, BC[f'{name}{i}']] = p[name][i]
    for i in range(D):
        Bf[:, BC[f'Whc1{i}']] = p['Whc1'][i][:, 0]
        Bf[:, BC[f'Whp1{i}']] = p['Whp1'][i][:, 0]
        Bf[:, BC[f'bhp1{i}']] = p['bhp1'][i][0]
        Bf[:, BC[f'bhc1{i}']] = p['bhc1'][i][0]
    Bf[:, BC['bh_r']] = 0.5 * (p['bih'][0:128] + p['bhh'][0:128])
    Bf[:, BC['bh_z']] = 0.5 * (p['bih'][128:256] + p['bhh'][128:256])
    Bf[:, BC['bih_n']] = p['bih'][256:384]
    Bf[:, BC['bhh_n']] = p['bhh'][256:384]
    Bf[:, BC['b_out']] = p['b_out'][0]

    Wo = np.ascontiguousarray(p['W_out'].reshape(H, H))
    return Wr, Bf, Wo


def kernel(comp_feature, prot_feature, batch_comp, batch_prot, params):
    from concourse.bass_utils import run_bass_kernel_spmd

    if 'nc' not in _cache:
        _cache['nc'], _cache['WC'], _cache['BC'], _cache['NW'], _cache['NB'] = _build()
    nc = _cache['nc']

    comp = np.asarray(comp_feature, dtype=np.float32)
    prot = np.asarray(prot_feature, dtype=np.float32)
    Wr, Bf, Wo = _host_pack(params, _cache['WC'], _cache['BC'],
                            _cache['NW'], _cache['NB'])

    comp3 = comp.reshape(B, LC, 128)
    prot3 = prot.reshape(B, LP, 128)
    in_maps = []
    for c in range(NCORES):
        gs = slice(c * G, (c + 1) * G)
        compT = np.ascontiguousarray(
            comp3[gs].transpose(2, 0, 1).reshape(128, G * LC))
        protT = np.ascontiguousarray(
            prot3[gs].transpose(2, 0, 1).reshape(128, G * LP))
        in_maps.append({'protT': protT, 'compT': compT,
                        'Wr': Wr, 'Bf': Bf, 'Wo': Wo})

    res = run_bass_kernel_spmd(nc, in_maps, core_ids=list(range(NCORES)))
    out = np.concatenate([res.results[c]['out'].reshape(G) for c in range(NCORES)])
    return out.reshape(B, 1).astype(np.float32)
